# revision 16
# baseline (speedup 1.0000x reference)
"""Trainium2 Bass kernel for nn_CrossAttention (B=2, Nq=Nk=2048, H=8, Dh=64,
Dx=512, Dctx=768).

Sharding: (batch, head-pair) across 8 cores — core c = (batch c//4, head pair
c%4). Each core computes K/V/Q projections for its 2 heads only, full
attention over 2048x2048 for those heads, and a PARTIAL output projection
(rows hp*128:(hp+1)*128 of Wo). The host sums the 4 bf16 partials per batch
at gather time (row-parallel Wo).

v3 vs the 132.9us baseline:
- QK runs with TRUE K=64 contraction, row-tiled: head A's stationary lives in
  PE rows 0:63 (tile_position (0,0)), head B's in rows 64:127 ((64,0)); the
  two matmuls execute CONCURRENTLY in the array, halving QK wall time vs the
  zero-padded 128-mode.  kt is one [128, NKV] tile (A dh on partitions 0:64,
  B on 64:128) straight out of the kproj psum — no zero-pad memsets.
- The softmax reciprocal no longer touches the scalar engine: the raw
  denominators are broadcast with two K=1 PE matmuls (selector tile picks
  den row 64 of the psA copy / row 32 of psB), one vector-engine
  reciprocal_approx_fast [128,512] inverts the broadcast, two tensor_tensor
  mults normalize.  Scalar does exp only: 64 x [128,1024].
- Every dma_start costs ~600ns of serial DIRECT2D issue on the Sync engine,
  so transfers are MERGED via 3D access patterns: one packed weights blob,
  ctx in 3-chunk x kv-half groups, xt split q-chunk-0 / rest, one store
  trigger per qc (4 output slices each).  8 input + 4 output triggers vs 38.
  The first QK pair needs only weights + ctx kv-half 0 + xt cols 0:512
  (2.64MB), so EXP starts ~14us earlier.
- kproj kv-half 0 accumulates in the score psum banks before attention;
  kv-half 1 in the acc pool during early QK pairs.
- Output partials are stored bf16, host sums in f32 (emulated end-to-end
  numerics: rel err 0.0074 vs 2e-2 budget).
"""

import sys

sys.path.insert(0, "/opt/trn_rl_repo")

import numpy as np
import ml_dtypes

import concourse.bacc as bacc
import concourse.mybir as mybir
import concourse.tile as tile
from concourse.bass_utils import run_bass_kernel_spmd
from contextlib import ExitStack

F32 = mybir.dt.float32
BF16 = mybir.dt.bfloat16
NP_BF16 = np.dtype(ml_dtypes.bfloat16)

B = 2
NQ = 2048
NKV = 2048
DX = 512
DC = 768
DI = 512
NH = 8
DH = 64
N_CORES = 8

KC_X = DX // 128  # 4 contraction chunks for x
KC_C = DC // 128  # 6 contraction chunks for context
MO = DI // 128  # 4 output-row chunks
NKC = NKV // 128  # 16 kv chunks
NQC = NQ // 512  # 4 q chunks of 512
NPAIR = NKC // 2  # 8 kv chunk-pairs per q chunk
SCALE = DH ** -0.5
DW_ALL = KC_C * 128 + KC_X * 128 + KC_C * 128 + DI  # wk|wq|wv|wo = 2560

_CACHE = {}


def _build_nc():
    nc = bacc.Bacc("TRN2", target_bir_lowering=False, debug=False, num_devices=N_CORES)

    xt = nc.declare_dram_parameter("xt", [DX, NQ], BF16, isOutput=False)
    ctxt = nc.declare_dram_parameter("ctxt", [DC, NKV], BF16, isOutput=False)
    w_all = nc.declare_dram_parameter("w_all", [128, DW_ALL], BF16, isOutput=False)
    ot = nc.declare_dram_parameter("ot", [DI, NQ], BF16, isOutput=True)

    # 3D views for merged DMA: chunk-major DRAM rows -> [partition, chunk, col]
    ctx3d = ctxt.rearrange("(c p) n -> p c n", p=128)
    xt3d = xt.rearrange("(c p) n -> p c n", p=128)
    ot3d = ot.rearrange("(m p) n -> p m n", p=128)

    with tile.TileContext(nc) as tc:
        with ExitStack() as ctx:
            # ---- SBUF pools ----
            const_p = ctx.enter_context(tc.tile_pool(name="const", bufs=1))
            w_p = ctx.enter_context(tc.tile_pool(name="weights", bufs=1))
            ctx_p = ctx.enter_context(tc.tile_pool(name="ctxt", bufs=1))
            xt_p = ctx.enter_context(tc.tile_pool(name="xt", bufs=1))
            kt_p = ctx.enter_context(tc.tile_pool(name="kt", bufs=1))
            qt_p = ctx.enter_context(tc.tile_pool(name="qt", bufs=1))
            vaug_p = ctx.enter_context(tc.tile_pool(name="vaug", bufs=1))
            p_p = ctx.enter_context(tc.tile_pool(name="pp", bufs=3))
            at_p = ctx.enter_context(tc.tile_pool(name="at", bufs=2))
            out_p = ctx.enter_context(tc.tile_pool(name="outsb", bufs=2))
            # ---- PSUM pools: 4 (scores) + 2 (attn) + 2 (acc) = 8 banks ----
            acc_ps = ctx.enter_context(tc.tile_pool(name="acc_ps", bufs=2, space="PSUM"))
            s_ps = ctx.enter_context(tc.tile_pool(name="s_ps", bufs=1, space="PSUM"))
            attn_ps = ctx.enter_context(tc.tile_pool(name="attn_ps", bufs=1, space="PSUM"))

            # Only Exp runs on the scalar engine now; pin its table set once.
            nc.scalar.add_instruction(
                mybir.InstLoadActFuncSet(
                    name=nc.get_next_instruction_name(),
                    act_func_set_id=6, ins=[], outs=[]))

            # ---- constants ----
            # e_tile: bcast selector for the RAW denominators. Row 64 serves
            # head A (cols 0:64), row 32 serves head B (cols 64:128).
            e_tile = const_p.tile([128, 128], BF16)
            nc.any.memset(e_tile[:], 0.0)
            nc.any.memset(e_tile[64:65, 0:64], 1.0)
            nc.any.memset(e_tile[32:33, 64:128], 1.0)

            # ---- DMA inputs: few triggers, ordered by the critical path to
            # the first EXP (weights, ctx kv-half 0, xt q-chunk 0), with the
            # rest streaming behind. ----
            w_sb = w_p.tile([128, DW_ALL], BF16, name="w_sb")
            ctx_b = ctx_p.tile([128, KC_C * NKV], BF16, name="ctx_b")
            xt_b = xt_p.tile([128, KC_X * NQ], BF16, name="xt_b")

            # ctx_b is kv-QUARTER-major (quarter g: cols [g*3072,(g+1)*3072),
            # chunk c at +c*512) and xt_b is q-block-major (q 0:512: cols
            # [0,2048) chunk-major 512 each; q 512:2048: cols [2048,8192)
            # chunk-major 1536 each), so every transfer writes a CONTIGUOUS
            # column interval of its tile — the tile dependency tracker is
            # interval-based, and a strided write would false-couple every
            # later reader to the last transfer.
            def dma_ctx_quarter(g):
                nc.sync.dma_start(
                    ctx_b[:, g * 3072:(g + 1) * 3072].rearrange(
                        "p (c n) -> p c n", n=512),
                    ctx3d[:, :, g * 512:(g + 1) * 512])

            nc.sync.dma_start(w_sb[:], w_all[:, :])
            dma_ctx_quarter(0)
            dma_ctx_quarter(1)
            nc.sync.dma_start(
                xt_b[:, 0:2048].rearrange("p (c n) -> p c n", n=512),
                xt3d[:, :, 0:512])
            nc.sync.dma_start(
                xt_b[:, 2048:8192].rearrange("p (c n) -> p c n", n=1536),
                xt3d[:, :, 512:2048])
            dma_ctx_quarter(2)
            dma_ctx_quarter(3)

            wk_t = [w_sb[:, c * 128:(c + 1) * 128] for c in range(KC_C)]
            _o = KC_C * 128
            wq_t = [w_sb[:, _o + c * 128:_o + (c + 1) * 128] for c in range(KC_X)]
            _o += KC_X * 128
            wv_t = [w_sb[:, _o + c * 128:_o + (c + 1) * 128] for c in range(KC_C)]
            _o += KC_C * 128
            wo_t = [w_sb[:, _o + m * 128:_o + (m + 1) * 128] for m in range(MO)]

            def ctx_c(c, lo, ln):
                # ctx kv cols [lo, lo+ln) of chunk c; must stay in one quarter
                g = lo // 512
                off = g * 3072 + c * 512 + (lo - g * 512)
                return ctx_b[:, off:off + ln]

            def xt_c(c, lo, ln):
                # x q cols [lo, lo+ln) of chunk c; q0 block or the rest
                if lo < 512:
                    off = c * 512 + lo
                else:
                    off = 2048 + c * 1536 + (lo - 512)
                return xt_b[:, off:off + ln]

            # ---- persistent activation tiles ----
            kt = kt_p.tile([128, NKV], BF16, name="kt")
            qt = qt_p.tile([128, NQ], BF16, name="qt")
            # vaug: per-kv-chunk layout [65 for head A | 128 for B].
            # A = 64 V + ones col -> attn_A on psum partitions 0:64, denom_A
            # on 64. B = [zeros:32 | ones | zeros:31 | 64 V] -> attn_B on
            # psum partitions 64:128, denom_B on partition 32.
            WB = 128
            WC = 65 + WB  # 193 per chunk
            va = vaug_p.tile([128, NKC * WC + 65], BF16, name="va")
            va3 = va[:, 0:NKC * WC].rearrange("p (g c) -> p g c", c=WC)
            nc.any.memset(va3[:, :, 64:65], 1.0)       # A ones col
            nc.any.memset(va3[:, :, 65:65 + 32], 0.0)  # B pad
            nc.any.memset(va3[:, :, 65 + 32:65 + 33], 1.0)  # B ones col
            nc.any.memset(va3[:, :, 65 + 33:65 + 64], 0.0)  # B pad

            # ---- K projection: kv-half 0 in the score psum banks before
            # attention starts, kv-half 1 in the acc pool during early QK
            # pairs (only gates pair 4+). ----
            def emit_kproj_h0():
                kp = s_ps.tile([128, 1024], F32, tag="s0", name="kp01")
                for g in range(2):
                    for c in range(KC_C):
                        nc.tensor.matmul(
                            kp[:, g * 512:(g + 1) * 512], wk_t[c],
                            ctx_c(c, g * 512, 512),
                            start=(c == 0), stop=(c == KC_C - 1))
                for g in range(2):
                    gs = slice(g * 512, (g + 1) * 512)
                    nc.vector.tensor_copy(kt[:, gs], kp[:, gs])

            def emit_kproj_h1():
                for g in range(2, 4):
                    ps = acc_ps.tile([128, 512], F32, tag="acc", name=f"kp{g}")
                    for c in range(KC_C):
                        nc.tensor.matmul(
                            ps[:], wk_t[c], ctx_c(c, g * 512, 512),
                            start=(c == 0), stop=(c == KC_C - 1))
                    nc.vector.tensor_copy(kt[:, g * 512:(g + 1) * 512], ps[:])

            def emit_qproj(n):
                ps = acc_ps.tile([128, 512], F32, tag="acc", name=f"pq{n}")
                for c in range(KC_X):
                    nc.tensor.matmul(
                        ps[:], wq_t[c], xt_c(c, n * 512, 512),
                        start=(c == 0), stop=(c == KC_X - 1))
                nc.vector.tensor_copy(qt[:, n * 512:(n + 1) * 512], ps[:])

            # ---- V projection, 4 kv chunks per psum tile; two strided
            # copies evacuate all 4 chunks x both heads ----
            def emit_v4(g):
                ps = acc_ps.tile([128, 512], F32, tag="acc", name=f"pv{g}")
                for j in range(4):
                    kvc = g * 4 + j
                    for c in range(KC_C):
                        nc.tensor.matmul(
                            ps[:, j * 128:(j + 1) * 128],
                            ctx_c(c, kvc * 128, 128), wv_t[c],
                            start=(c == 0), stop=(c == KC_C - 1))
                dst = va[:, g * 4 * WC:(g + 1) * 4 * WC].rearrange(
                    "p (c r) -> p c r", r=WC)
                src = ps[:].rearrange("p (c r) -> p c r", r=128)
                nc.vector.tensor_copy(dst[:, :, 0:64], src[:, :, 0:64])
                nc.vector.tensor_copy(dst[:, :, 129:193], src[:, :, 64:128])

            # ---- attention ----
            psa = {}  # (head) -> live attn psum tile
            psa_sb = {}  # (head) -> SBUF copy of attn + denom
            p_ts = {}  # (head, pair) -> P tile
            at_tiles = [None] * NQC

            def emit_qk(h, qc, p):
                # True K=64, row-tiled: head A in PE rows 0:63, head B in
                # 64:127 — base_partition auto-derives tile_position; the two
                # heads' matmuls run concurrently in the array.
                hs = slice(h * 64, (h + 1) * 64)
                ps_s = s_ps.tile([128, 1024], F32, tag=f"s{h}", name=f"s{h}_{qc}_{p}")
                for j in range(2):
                    kvc = p * 2 + j
                    nc.tensor.matmul(
                        ps_s[:, j * 512:(j + 1) * 512],
                        kt[hs, kvc * 128:(kvc + 1) * 128],
                        qt[hs, qc * 512:(qc + 1) * 512],
                        start=True, stop=True)
                p_t = p_p.tile([128, 1024], BF16, tag=f"p{h}", name=f"p{h}_{qc}_{p}")
                nc.scalar.activation(p_t[:], ps_s[:],
                                     mybir.ActivationFunctionType.Exp, scale=SCALE)
                p_ts[(h, p)] = p_t

            def emit_pv(h, qc, p):
                w = 65 if h == 0 else WB
                off = 0 if h == 0 else 65
                if p == 0:
                    psa[h] = attn_ps.tile([w, 512], F32, tag=f"a{h}",
                                          name=f"a{h}_{qc}")
                for j in range(2):
                    kvc = p * 2 + j
                    nc.tensor.matmul(
                        psa[h][:], va[:, kvc * WC + off:kvc * WC + off + w],
                        p_ts[(h, p)][:, j * 512:(j + 1) * 512],
                        start=(kvc == 0), stop=(kvc == NKC - 1))

            # normalize pipeline, spread across the next qc's pair iters.
            def emit_psevac(qc):
                # den rows first so the bcast matmuls can fire while the big
                # B-attn copy is still draining
                tA = at_p.tile([65, 512], BF16, tag="psA", name=f"psA{qc}")
                nc.vector.tensor_copy(tA[:], psa[0][:])
                psa_sb[0] = tA
                tB = at_p.tile([128, 512], BF16, tag="psB", name=f"psB{qc}")
                nc.vector.tensor_copy(tB[32:33, :], psa[1][32:33, :])
                nc.vector.tensor_copy(tB[64:128, :], psa[1][64:128, :])
                psa_sb[1] = tB

            def emit_bcast(qc):
                # ps_b rows 0:64 <- d_A (tA row 64), rows 64:128 <- d_B (tB
                # row 32). Two K=1 matmuls contracting over ONLY the den row
                # — garbage rows in tA/tB are never touched (NaN * 0.0 would
                # poison a full-K contraction).
                ps_b = acc_ps.tile([128, 512], F32, tag="acc", name=f"bc{qc}")
                nc.tensor.matmul(ps_b[0:64, :], e_tile[64:65, 0:64],
                                 psa_sb[0][64:65, :], start=True, stop=True)
                nc.tensor.matmul(ps_b[64:128, :], e_tile[32:33, 64:128],
                                 psa_sb[1][32:33, :], start=True, stop=True)
                return ps_b

            def emit_recip(qc, ps_b):
                bc_sb = at_p.tile([128, 512], F32, tag="bc", name=f"bcs{qc}")
                nc.vector.reciprocal_approx_fast(out=bc_sb[:], in_=ps_b[:])
                return bc_sb

            def emit_atmult(qc, bc_sb):
                a_t = at_p.tile([128, 512], BF16, tag="at", name=f"at{qc}")
                nc.vector.tensor_tensor(a_t[0:64, :], psa_sb[0][0:64, :],
                                        bc_sb[0:64, :], op=mybir.AluOpType.mult)
                nc.vector.tensor_tensor(a_t[64:128, :], psa_sb[1][64:128, :],
                                        bc_sb[64:128, :], op=mybir.AluOpType.mult)
                at_tiles[qc] = a_t

            def emit_oproj(qc):
                # mid-flight qcs: one store trigger for all 4 output-row
                # chunks (sync DIRECT2D issue is ~600ns each). Last qc: per-m
                # stores so the drain overlaps the remaining oproj matmuls.
                split = qc == NQC - 1
                o_sb = out_p.tile([128, MO * 512], BF16, tag="osb", name=f"ob{qc}")
                for m in range(MO):
                    ps = acc_ps.tile([128, 512], F32, tag="acc", name=f"o{qc}_{m}")
                    nc.tensor.matmul(ps[:], wo_t[m], at_tiles[qc][:],
                                     start=True, stop=True)
                    nc.vector.tensor_copy(o_sb[:, m * 512:(m + 1) * 512], ps[:])
                    if split:
                        nc.sync.dma_start(
                            ot[m * 128:(m + 1) * 128, qc * 512:(qc + 1) * 512],
                            o_sb[:, m * 512:(m + 1) * 512])
                if not split:
                    nc.sync.dma_start(
                        ot3d[:, :, qc * 512:(qc + 1) * 512],
                        o_sb[:].rearrange("p (m n) -> p m n", n=512))

            emit_kproj_h0()
            emit_qproj(0)
            pend_bc = None
            pend_rec = None
            for qc in range(NQC):
                for p in range(NPAIR + 1):
                    if p < NPAIR:
                        emit_qk(0, qc, p)
                        emit_qk(1, qc, p)
                    if qc == 0:
                        if p == 0:
                            emit_v4(0)
                        elif p == 1:
                            emit_kproj_h1()
                        elif p == 2:
                            emit_v4(1)
                        elif p == 3:
                            emit_qproj(1)
                        elif p == 4:
                            emit_v4(2)
                        elif p == 5:
                            emit_v4(3)
                    if qc in (1, 2) and p == 5:
                        emit_qproj(qc + 1)
                    if qc > 0:
                        # spread the previous qc's normalize chain so no
                        # single engine sees a block of serial work
                        if p == 0:
                            emit_psevac(qc - 1)
                        elif p == 1:
                            pend_bc = emit_bcast(qc - 1)
                        elif p == 2:
                            pend_rec = emit_recip(qc - 1, pend_bc)
                        elif p == 3:
                            emit_atmult(qc - 1, pend_rec)
                        elif p == 4:
                            emit_oproj(qc - 1)
                    if p >= 1:
                        emit_pv(0, qc, p - 1)
                        emit_pv(1, qc, p - 1)
            emit_psevac(NQC - 1)
            pend_bc = emit_bcast(NQC - 1)
            pend_rec = emit_recip(NQC - 1, pend_bc)
            emit_atmult(NQC - 1, pend_rec)
            emit_oproj(NQC - 1)

    nc.finalize()
    return nc


def _bf16(a):
    return np.ascontiguousarray(a).astype(NP_BF16)


def run_spmd(inputs, trace=False):
    if "nc" not in _CACHE:
        _CACHE["nc"] = _build_nc()
    nc = _CACHE["nc"]

    x = np.asarray(inputs["x"], dtype=np.float32)
    context = np.asarray(inputs["context"], dtype=np.float32)
    wq_f = np.asarray(inputs["Wq"], np.float32)
    wk_f = np.asarray(inputs["Wk"], np.float32)
    wv_f = np.asarray(inputs["Wv"], np.float32)
    wo_f = np.asarray(inputs["Wo"], np.float32)
    bo_f = np.asarray(inputs["bo"], np.float32)

    def pack(w):
        # [K*128, 128] -> [128, K*128]: row p holds chunk-c columns side by
        # side, so one 128-row DMA carries all contraction chunks
        k = w.shape[0] // 128
        return w.reshape(k, 128, 128).transpose(1, 0, 2).reshape(128, k * 128)

    xt_b = [_bf16(x[b].T) for b in range(B)]
    ctxt_b = [_bf16(context[b].T) for b in range(B)]
    in_maps = []
    for c in range(N_CORES):
        b, hp = c // 4, c % 4
        cs = slice(hp * 128, (hp + 1) * 128)
        w_parts = np.concatenate(
            [pack(wk_f[:, cs]), pack(wq_f[:, cs]), pack(wv_f[:, cs]),
             wo_f[cs, :]], axis=1)
        in_maps.append({
            "xt": xt_b[b], "ctxt": ctxt_b[b], "w_all": _bf16(w_parts),
        })

    res = run_bass_kernel_spmd(nc, in_maps, core_ids=list(range(N_CORES)),
                               trace=trace)
    out = np.empty((B, NQ, DI), dtype=np.float32)
    for b in range(B):
        acc = res.results[b * 4]["ot"].astype(np.float32)
        for hp in range(1, 4):
            acc = acc + res.results[b * 4 + hp]["ot"].astype(np.float32)
        out[b] = acc.T + bo_f[None, :]
    return out, res


def kernel(**inputs):
    out, _ = run_spmd(inputs, trace=False)
    return out


# revision 19
# speedup vs baseline: 1.1755x; 1.1755x over previous
"""Trainium2 Bass kernel for nn_CrossAttention (B=2, Nq=Nk=2048, H=8, Dh=64,
Dx=512, Dctx=768).

Sharding: (batch, head-pair) across 8 cores — core c = (batch c//4, head pair
c%4). Each core computes K/V/Q projections for its 2 heads only, full
attention over 2048x2048 for those heads, and a PARTIAL output projection
(rows hp*128:(hp+1)*128 of Wo). The host sums the 4 bf16 partials per batch
at gather time (row-parallel Wo).

v3 vs the 132.9us baseline:
- QK runs with TRUE K=64 contraction, row-tiled: head A's stationary lives in
  PE rows 0:63 (tile_position (0,0)), head B's in rows 64:127 ((64,0)); the
  two matmuls execute CONCURRENTLY in the array, halving QK wall time vs the
  zero-padded 128-mode.  kt is one [128, NKV] tile (A dh on partitions 0:64,
  B on 64:128) straight out of the kproj psum — no zero-pad memsets.
- The softmax reciprocal no longer touches the scalar engine: the raw
  denominators are broadcast with two K=1 PE matmuls (selector tile picks
  den row 64 of the psA copy / row 32 of psB), one vector-engine
  reciprocal_approx_fast [128,512] inverts the broadcast, two tensor_tensor
  mults normalize.  Scalar does exp only: 64 x [128,1024].
- Every dma_start costs ~600ns of serial DIRECT2D issue on the Sync engine,
  so transfers are MERGED via 3D access patterns: one packed weights blob,
  ctx in 3-chunk x kv-half groups, xt split q-chunk-0 / rest, one store
  trigger per qc (4 output slices each).  8 input + 4 output triggers vs 38.
  The first QK pair needs only weights + ctx kv-half 0 + xt cols 0:512
  (2.64MB), so EXP starts ~14us earlier.
- kproj kv-half 0 accumulates in the score psum banks before attention;
  kv-half 1 in the acc pool during early QK pairs.
- Output partials are stored bf16, host sums in f32 (emulated end-to-end
  numerics: rel err 0.0074 vs 2e-2 budget).
"""

import sys

sys.path.insert(0, "/opt/trn_rl_repo")

import numpy as np
import ml_dtypes

import concourse.bacc as bacc
import concourse.mybir as mybir
import concourse.tile as tile
from concourse.bass_utils import run_bass_kernel_spmd
from contextlib import ExitStack

F32 = mybir.dt.float32
BF16 = mybir.dt.bfloat16
NP_BF16 = np.dtype(ml_dtypes.bfloat16)

B = 2
NQ = 2048
NKV = 2048
DX = 512
DC = 768
DI = 512
NH = 8
DH = 64
N_CORES = 8

KC_X = DX // 128  # 4 contraction chunks for x
KC_C = DC // 128  # 6 contraction chunks for context
MO = DI // 128  # 4 output-row chunks
NKC = NKV // 128  # 16 kv chunks
NQC = NQ // 512  # 4 q chunks of 512
NPAIR = NKC // 2  # 8 kv chunk-pairs per q chunk
SCALE = DH ** -0.5
DW_ALL = KC_C * 128 + KC_X * 128 + KC_C * 128 + DI  # wk|wq|wv|wo = 2560

_CACHE = {}


def _build_nc():
    nc = bacc.Bacc("TRN2", target_bir_lowering=False, debug=False, num_devices=N_CORES)

    xt = nc.declare_dram_parameter("xt", [DX, NQ], BF16, isOutput=False)
    ctxt = nc.declare_dram_parameter("ctxt", [DC, NKV], BF16, isOutput=False)
    w_all = nc.declare_dram_parameter("w_all", [128, DW_ALL], BF16, isOutput=False)
    ot = nc.declare_dram_parameter("ot", [DI, NQ], BF16, isOutput=True)

    # 3D views for merged DMA: chunk-major DRAM rows -> [partition, chunk, col]
    ctx3d = ctxt.rearrange("(c p) n -> p c n", p=128)
    xt3d = xt.rearrange("(c p) n -> p c n", p=128)
    ot3d = ot.rearrange("(m p) n -> p m n", p=128)

    with tile.TileContext(nc) as tc:
        with ExitStack() as ctx:
            # ---- SBUF pools ----
            const_p = ctx.enter_context(tc.tile_pool(name="const", bufs=1))
            w_p = ctx.enter_context(tc.tile_pool(name="weights", bufs=1))
            ctx_p = ctx.enter_context(tc.tile_pool(name="ctxt", bufs=1))
            xt_p = ctx.enter_context(tc.tile_pool(name="xt", bufs=1))
            kt_p = ctx.enter_context(tc.tile_pool(name="kt", bufs=1))
            qt_p = ctx.enter_context(tc.tile_pool(name="qt", bufs=1))
            vaug_p = ctx.enter_context(tc.tile_pool(name="vaug", bufs=1))
            p_p = ctx.enter_context(tc.tile_pool(name="pp", bufs=3))
            at_p = ctx.enter_context(tc.tile_pool(name="at", bufs=2))
            out_p = ctx.enter_context(tc.tile_pool(name="outsb", bufs=2))
            # ---- PSUM pools: 4 (scores) + 2 (attn) + 2 (acc) = 8 banks ----
            acc_ps = ctx.enter_context(tc.tile_pool(name="acc_ps", bufs=2, space="PSUM"))
            s_ps = ctx.enter_context(tc.tile_pool(name="s_ps", bufs=1, space="PSUM"))
            attn_ps = ctx.enter_context(tc.tile_pool(name="attn_ps", bufs=1, space="PSUM"))

            # Only Exp runs on the scalar engine now; pin its table set once.
            nc.scalar.add_instruction(
                mybir.InstLoadActFuncSet(
                    name=nc.get_next_instruction_name(),
                    act_func_set_id=6, ins=[], outs=[]))

            # ---- constants ----
            # e_tile: bcast selector for the RAW denominators. Row 64 serves
            # head A (cols 0:64), row 32 serves head B (cols 64:128).
            e_tile = const_p.tile([128, 128], BF16)
            nc.any.memset(e_tile[:], 0.0)
            nc.any.memset(e_tile[64:65, 0:64], 1.0)
            nc.any.memset(e_tile[32:33, 64:128], 1.0)

            # ---- DMA inputs: few triggers, ordered by the critical path to
            # the first EXP (weights, ctx kv-half 0, xt q-chunk 0), with the
            # rest streaming behind. ----
            # One SBUF tile PER TRANSFER: a multi-dim DMA write is tracked at
            # whole-tile granularity, so readers of a shared big tile would
            # falsely wait on every later transfer into it.
            w_sb = w_p.tile([128, DW_ALL], BF16, name="w_sb")
            ctxq = [ctx_p.tile([128, KC_C * 512], BF16, tag=f"cq{g}",
                               name=f"cq{g}") for g in range(4)]
            xtq0 = xt_p.tile([128, KC_X * 512], BF16, tag="xq0", name="xq0")
            xtr = xt_p.tile([128, KC_X * 1536], BF16, tag="xtr", name="xtr")

            # ctx_b is kv-QUARTER-major (quarter g: cols [g*3072,(g+1)*3072),
            # chunk c at +c*512) and xt_b is q-block-major (q 0:512: cols
            # [0,2048) chunk-major 512 each; q 512:2048: cols [2048,8192)
            # chunk-major 1536 each), so every transfer writes a CONTIGUOUS
            # column interval of its tile — the tile dependency tracker is
            # interval-based, and a strided write would false-couple every
            # later reader to the last transfer.
            def dma_ctx_quarter(g):
                nc.sync.dma_start(
                    ctxq[g][:].rearrange("p (c n) -> p c n", n=512),
                    ctx3d[:, :, g * 512:(g + 1) * 512])

            nc.sync.dma_start(w_sb[:], w_all[:, :])
            dma_ctx_quarter(0)
            dma_ctx_quarter(1)
            nc.sync.dma_start(
                xtq0[:].rearrange("p (c n) -> p c n", n=512),
                xt3d[:, :, 0:512])
            nc.sync.dma_start(
                xtr[:].rearrange("p (c n) -> p c n", n=1536),
                xt3d[:, :, 512:2048])
            dma_ctx_quarter(2)
            dma_ctx_quarter(3)

            wk_t = [w_sb[:, c * 128:(c + 1) * 128] for c in range(KC_C)]
            _o = KC_C * 128
            wq_t = [w_sb[:, _o + c * 128:_o + (c + 1) * 128] for c in range(KC_X)]
            _o += KC_X * 128
            wv_t = [w_sb[:, _o + c * 128:_o + (c + 1) * 128] for c in range(KC_C)]
            _o += KC_C * 128
            wo_t = [w_sb[:, _o + m * 128:_o + (m + 1) * 128] for m in range(MO)]

            def ctx_c(c, lo, ln):
                # ctx kv cols [lo, lo+ln) of chunk c; must stay in one quarter
                g = lo // 512
                off = c * 512 + (lo - g * 512)
                return ctxq[g][:, off:off + ln]

            def xt_c(c, lo, ln):
                # x q cols [lo, lo+ln) of chunk c; q0 block or the rest
                if lo < 512:
                    return xtq0[:, c * 512 + lo:c * 512 + lo + ln]
                off = c * 1536 + (lo - 512)
                return xtr[:, off:off + ln]

            # ---- persistent activation tiles ----
            kt = kt_p.tile([128, NKV], BF16, name="kt")
            qt = qt_p.tile([128, NQ], BF16, name="qt")
            # vaug: per-kv-chunk layout [65 for head A | 128 for B].
            # A = 64 V + ones col -> attn_A on psum partitions 0:64, denom_A
            # on 64. B = [zeros:32 | ones | zeros:31 | 64 V] -> attn_B on
            # psum partitions 64:128, denom_B on partition 32.
            WB = 128
            WC = 65 + WB  # 193 per chunk
            va = vaug_p.tile([128, NKC * WC + 65], BF16, name="va")
            va3 = va[:, 0:NKC * WC].rearrange("p (g c) -> p g c", c=WC)
            nc.any.memset(va3[:, :, 64:65], 1.0)       # A ones col
            nc.any.memset(va3[:, :, 65:65 + 32], 0.0)  # B pad
            nc.any.memset(va3[:, :, 65 + 32:65 + 33], 1.0)  # B ones col
            nc.any.memset(va3[:, :, 65 + 33:65 + 64], 0.0)  # B pad

            # ---- K projection: kv-half 0 in the score psum banks before
            # attention starts, kv-half 1 in the acc pool during early QK
            # pairs (only gates pair 4+). ----
            def emit_kproj_h0():
                kp = s_ps.tile([128, 1024], F32, tag="s0", name="kp01")
                for g in range(2):
                    for c in range(KC_C):
                        nc.tensor.matmul(
                            kp[:, g * 512:(g + 1) * 512], wk_t[c],
                            ctx_c(c, g * 512, 512),
                            start=(c == 0), stop=(c == KC_C - 1))
                for g in range(2):
                    gs = slice(g * 512, (g + 1) * 512)
                    nc.vector.tensor_copy(kt[:, gs], kp[:, gs])

            def emit_kproj_h1():
                for g in range(2, 4):
                    ps = acc_ps.tile([128, 512], F32, tag="acc", name=f"kp{g}")
                    for c in range(KC_C):
                        nc.tensor.matmul(
                            ps[:], wk_t[c], ctx_c(c, g * 512, 512),
                            start=(c == 0), stop=(c == KC_C - 1))
                    nc.vector.tensor_copy(kt[:, g * 512:(g + 1) * 512], ps[:])

            def emit_qproj(n):
                ps = acc_ps.tile([128, 512], F32, tag="acc", name=f"pq{n}")
                for c in range(KC_X):
                    nc.tensor.matmul(
                        ps[:], wq_t[c], xt_c(c, n * 512, 512),
                        start=(c == 0), stop=(c == KC_X - 1))
                nc.vector.tensor_copy(qt[:, n * 512:(n + 1) * 512], ps[:])

            # ---- V projection, 4 kv chunks per psum tile; two strided
            # copies evacuate all 4 chunks x both heads ----
            def emit_v4(g):
                ps = acc_ps.tile([128, 512], F32, tag="acc", name=f"pv{g}")
                for j in range(4):
                    kvc = g * 4 + j
                    for c in range(KC_C):
                        nc.tensor.matmul(
                            ps[:, j * 128:(j + 1) * 128],
                            ctx_c(c, kvc * 128, 128), wv_t[c],
                            start=(c == 0), stop=(c == KC_C - 1))
                dst = va[:, g * 4 * WC:(g + 1) * 4 * WC].rearrange(
                    "p (c r) -> p c r", r=WC)
                src = ps[:].rearrange("p (c r) -> p c r", r=128)
                nc.vector.tensor_copy(dst[:, :, 0:64], src[:, :, 0:64])
                nc.vector.tensor_copy(dst[:, :, 129:193], src[:, :, 64:128])

            # ---- attention ----
            psa = {}  # (head) -> live attn psum tile
            psa_sb = {}  # (head) -> SBUF copy of attn + denom
            p_ts = {}  # (head, pair) -> P tile
            at_tiles = [None] * NQC

            def emit_qk(h, qc, p):
                # True K=64, row-tiled: head A in PE rows 0:63, head B in
                # 64:127 — base_partition auto-derives tile_position; the two
                # heads' matmuls run concurrently in the array.
                hs = slice(h * 64, (h + 1) * 64)
                ps_s = s_ps.tile([128, 1024], F32, tag=f"s{h}", name=f"s{h}_{qc}_{p}")
                for j in range(2):
                    kvc = p * 2 + j
                    nc.tensor.matmul(
                        ps_s[:, j * 512:(j + 1) * 512],
                        kt[hs, kvc * 128:(kvc + 1) * 128],
                        qt[hs, qc * 512:(qc + 1) * 512],
                        start=True, stop=True)
                p_t = p_p.tile([128, 1024], BF16, tag=f"p{h}", name=f"p{h}_{qc}_{p}")
                nc.scalar.activation(p_t[:], ps_s[:],
                                     mybir.ActivationFunctionType.Exp, scale=SCALE)
                p_ts[(h, p)] = p_t

            def emit_pv(h, qc, p):
                w = 65 if h == 0 else WB
                off = 0 if h == 0 else 65
                if p == 0:
                    psa[h] = attn_ps.tile([w, 512], F32, tag=f"a{h}",
                                          name=f"a{h}_{qc}")
                for j in range(2):
                    kvc = p * 2 + j
                    nc.tensor.matmul(
                        psa[h][:], va[:, kvc * WC + off:kvc * WC + off + w],
                        p_ts[(h, p)][:, j * 512:(j + 1) * 512],
                        start=(kvc == 0), stop=(kvc == NKC - 1))

            # normalize pipeline, spread across the next qc's pair iters.
            def emit_psevac(qc):
                # den rows first so the bcast matmuls can fire while the big
                # B-attn copy is still draining
                tA = at_p.tile([65, 512], BF16, tag="psA", name=f"psA{qc}")
                nc.vector.tensor_copy(tA[:], psa[0][:])
                psa_sb[0] = tA
                tB = at_p.tile([128, 512], BF16, tag="psB", name=f"psB{qc}")
                nc.vector.tensor_copy(tB[32:33, :], psa[1][32:33, :])
                nc.vector.tensor_copy(tB[64:128, :], psa[1][64:128, :])
                psa_sb[1] = tB

            def emit_bcast(qc):
                # ps_b rows 0:64 <- d_A (tA row 64), rows 64:128 <- d_B (tB
                # row 32). Two K=1 matmuls contracting over ONLY the den row
                # — garbage rows in tA/tB are never touched (NaN * 0.0 would
                # poison a full-K contraction).
                ps_b = acc_ps.tile([128, 512], F32, tag="acc", name=f"bc{qc}")
                nc.tensor.matmul(ps_b[0:64, :], e_tile[64:65, 0:64],
                                 psa_sb[0][64:65, :], start=True, stop=True)
                nc.tensor.matmul(ps_b[64:128, :], e_tile[32:33, 64:128],
                                 psa_sb[1][32:33, :], start=True, stop=True)
                return ps_b

            def emit_recip(qc, ps_b):
                bc_sb = at_p.tile([128, 512], F32, tag="bc", name=f"bcs{qc}")
                nc.vector.reciprocal_approx_fast(out=bc_sb[:], in_=ps_b[:])
                return bc_sb

            def emit_atmult(qc, bc_sb):
                a_t = at_p.tile([128, 512], BF16, tag="at", name=f"at{qc}")
                nc.vector.tensor_tensor(a_t[0:64, :], psa_sb[0][0:64, :],
                                        bc_sb[0:64, :], op=mybir.AluOpType.mult)
                nc.vector.tensor_tensor(a_t[64:128, :], psa_sb[1][64:128, :],
                                        bc_sb[64:128, :], op=mybir.AluOpType.mult)
                at_tiles[qc] = a_t

            def emit_oproj(qc):
                # mid-flight qcs: one store trigger for all 4 output-row
                # chunks (sync DIRECT2D issue is ~600ns each). Last qc: per-m
                # stores so the drain overlaps the remaining oproj matmuls.
                split = qc == NQC - 1
                o_sb = out_p.tile([128, MO * 512], BF16, tag="osb", name=f"ob{qc}")
                for m in range(MO):
                    ps = acc_ps.tile([128, 512], F32, tag="acc", name=f"o{qc}_{m}")
                    nc.tensor.matmul(ps[:], wo_t[m], at_tiles[qc][:],
                                     start=True, stop=True)
                    nc.vector.tensor_copy(o_sb[:, m * 512:(m + 1) * 512], ps[:])
                    if split:
                        nc.sync.dma_start(
                            ot[m * 128:(m + 1) * 128, qc * 512:(qc + 1) * 512],
                            o_sb[:, m * 512:(m + 1) * 512])
                if not split:
                    nc.sync.dma_start(
                        ot3d[:, :, qc * 512:(qc + 1) * 512],
                        o_sb[:].rearrange("p (m n) -> p m n", n=512))

            emit_kproj_h0()
            emit_qproj(0)
            pend_bc = None
            pend_rec = None
            for qc in range(NQC):
                for p in range(NPAIR + 1):
                    if p < NPAIR:
                        emit_qk(0, qc, p)
                        emit_qk(1, qc, p)
                    if qc == 0:
                        if p == 0:
                            emit_v4(0)
                        elif p == 1:
                            emit_kproj_h1()
                        elif p == 2:
                            emit_v4(1)
                        elif p == 3:
                            emit_qproj(1)
                        elif p == 4:
                            emit_v4(2)
                        elif p == 5:
                            emit_v4(3)
                    if qc in (1, 2) and p == 5:
                        emit_qproj(qc + 1)
                    if qc > 0:
                        # spread the previous qc's normalize chain so no
                        # single engine sees a block of serial work
                        if p == 0:
                            emit_psevac(qc - 1)
                        elif p == 1:
                            pend_bc = emit_bcast(qc - 1)
                        elif p == 2:
                            pend_rec = emit_recip(qc - 1, pend_bc)
                        elif p == 3:
                            emit_atmult(qc - 1, pend_rec)
                        elif p == 4:
                            emit_oproj(qc - 1)
                    if p >= 1:
                        emit_pv(0, qc, p - 1)
                        emit_pv(1, qc, p - 1)
            emit_psevac(NQC - 1)
            pend_bc = emit_bcast(NQC - 1)
            pend_rec = emit_recip(NQC - 1, pend_bc)
            emit_atmult(NQC - 1, pend_rec)
            emit_oproj(NQC - 1)

    nc.finalize()
    return nc


def _bf16(a):
    return np.ascontiguousarray(a).astype(NP_BF16)


def run_spmd(inputs, trace=False):
    if "nc" not in _CACHE:
        _CACHE["nc"] = _build_nc()
    nc = _CACHE["nc"]

    x = np.asarray(inputs["x"], dtype=np.float32)
    context = np.asarray(inputs["context"], dtype=np.float32)
    wq_f = np.asarray(inputs["Wq"], np.float32)
    wk_f = np.asarray(inputs["Wk"], np.float32)
    wv_f = np.asarray(inputs["Wv"], np.float32)
    wo_f = np.asarray(inputs["Wo"], np.float32)
    bo_f = np.asarray(inputs["bo"], np.float32)

    def pack(w):
        # [K*128, 128] -> [128, K*128]: row p holds chunk-c columns side by
        # side, so one 128-row DMA carries all contraction chunks
        k = w.shape[0] // 128
        return w.reshape(k, 128, 128).transpose(1, 0, 2).reshape(128, k * 128)

    xt_b = [_bf16(x[b].T) for b in range(B)]
    ctxt_b = [_bf16(context[b].T) for b in range(B)]
    in_maps = []
    for c in range(N_CORES):
        b, hp = c // 4, c % 4
        cs = slice(hp * 128, (hp + 1) * 128)
        w_parts = np.concatenate(
            [pack(wk_f[:, cs]), pack(wq_f[:, cs]), pack(wv_f[:, cs]),
             wo_f[cs, :]], axis=1)
        in_maps.append({
            "xt": xt_b[b], "ctxt": ctxt_b[b], "w_all": _bf16(w_parts),
        })

    res = run_bass_kernel_spmd(nc, in_maps, core_ids=list(range(N_CORES)),
                               trace=trace)
    out = np.empty((B, NQ, DI), dtype=np.float32)
    for b in range(B):
        acc = res.results[b * 4]["ot"].astype(np.float32)
        for hp in range(1, 4):
            acc = acc + res.results[b * 4 + hp]["ot"].astype(np.float32)
        out[b] = acc.T + bo_f[None, :]
    return out, res


def kernel(**inputs):
    out, _ = run_spmd(inputs, trace=False)
    return out
